# revision 25
# baseline (speedup 1.0000x reference)
"""Deformable Conv2d Lite (K=3) on 8 Trainium2 NeuronCores.

Sharding: data-parallel over batch x image-half. Core n handles sample n//2,
image rows [64*(n%2), 64*(n%2)+64). Weight replicated.

Device pipeline per core (rel err ~6e-4):
  1. DVE stage A: from raw offsets compute, per (tap, pixel), a gather index
     into a row-pair-interleaved fp16 NHWC layout of a ZERO-FRAMED (+2 texel
     border) image (pair-row r of xpair holds padded rows r, r+1 at one col;
     idx = clamp(y0+2, 0, 130)*132 + clamp(x0+2, 0, 130)); because clamped
     out-of-range fetches land on zero texels, the 4 bilinear corner weights
     are the plain products wx{0,1}*wy{0,1} -- no edge eq-masking (floor is
     int-cast + is_gt since mod is not a valid HW ALU op). Weights land in
     w4d with 8 REAL duplicates along the last dim (see step 3).
  2. SWDGE dma_gather (fp16): one 512B descriptor per (tap, pixel) fetches
     the full 2x2 x 64ch bilinear patch from DRAM; 108 calls of <=768
     indices (descriptor-ring capacity) round-robined over 4 SWDGE queues.
  3. DVE: in-place weight multiply with the weight operand viewed as
     [.., (slot corner), 8, [1,8]] over the 8 real duplicates: a packed
     last dim hits the DVE fast path -- measured 1.26us vs 3.3us/op for the
     stride-0 [0,64] broadcast. Then one packed fp16 add presums the two
     col-corners (L+R), halving everything downstream. The remaining
     row-corner ADD is folded into the conv matmul contraction. (Tried and
     rejected: accumulating is_transpose matmuls -- PE transposes do NOT
     accumulate in PSUM, they overwrite.)
  4. PE: fp16 transposes of the presummed patches to channel-major
     (K = 2 row-corners x 64ch = 128 per tap), then conv matmuls with
     per-tap weight slabs (W_t.T stacked twice) accumulating f32 in PSUM
     over all 9 taps.
  5. ACT: one PSUM->SBUF copy per (chunk, tap) of the transposed tile, and
     final bias-add on the conv PSUM; DMA out.
"""

import sys

for _p in ("/opt/trn_rl_repo",):
    if _p not in sys.path:
        sys.path.insert(0, _p)

import numpy as np

import concourse.bass as bass
import concourse.tile as tile
from concourse import bacc, mybir
from concourse.bass_utils import run_bass_kernel_spmd

F32 = mybir.dt.float32
F16 = mybir.dt.float16
I16 = mybir.dt.int16
Alu = mybir.AluOpType
Act = mybir.ActivationFunctionType

B, C, H, W = 4, 64, 128, 128
OC, KK = 64, 9
HALF = H // 2            # rows per core
PIX = HALF * W           # 8192 pixels per core
NCHUNK = 4
CPIX = PIX // NCHUNK     # 2048 pixels per chunk
CSLOT = CPIX // 128      # 16 slots per chunk
SLOTS = PIX // 128       # 64
NPAIR = 5                # ceil(9/2) tap pairs

XROWS = 2 * PIX + 1      # interleaved pair-row count incl. pad (PAD=0 mode)
# PAD=2 mode: image zero-framed with 2 texels per side -> clamped OOB
# fetches read zeros, so bilinear weights need no edge masking.
WP = W + 4               # padded cols
GP = H + 3               # pair-row groups (g0 in 0..130)
XROWSP = GP * WP + 1     # 17293


import os


def build_program(loop_n: int = 0, ablate: str = ""):
    """Build the per-core Bass program. loop_n>0 wraps the body in a device
    For_i loop (for wall-clock timing); loop_n==0 emits the plain body.
    ablate: comma-set of {nogather, nodve, nope} for perf bisection."""
    abl = set(ablate.split(",")) if ablate else set()
    import os
    pad2 = os.environ.get("PAD", "2") == "2"
    wmul = os.environ.get("WMUL", "dup8")
    NDUP = {"dup8": 8, "dup4": 4, "dup": 2, "bcast": 2}[wmul]
    xrows = XROWSP if pad2 else XROWS
    nc = bacc.Bacc("TRN2", target_bir_lowering=False, debug=False, num_devices=8,
                   num_swdge_queues=int(os.environ.get("NSQ", "4")),
                   dynamic_dma_scratch_size=int(os.environ.get("DDS", "16384")))

    xp = nc.dram_tensor("xpair", [xrows, 128], F16, kind="ExternalInput").ap()
    offs = nc.dram_tensor("offs", [128, SLOTS * 2 * KK], F32, kind="ExternalInput").ap()
    wp = nc.dram_tensor("wpair", [128, KK * OC], F16, kind="ExternalInput").ap()
    yyd = nc.dram_tensor("yy", [128, SLOTS], F32, kind="ExternalInput").ap()
    xxd = nc.dram_tensor("xx", [128, 1], F32, kind="ExternalInput").ap()
    idd = nc.dram_tensor("ident", [128, 128], F16, kind="ExternalInput").ap()
    bsd = nc.dram_tensor("bias", [OC, 1], F32, kind="ExternalInput").ap()
    out = nc.dram_tensor("out", [OC, PIX], F32, kind="ExternalOutput").ap()

    with tile.TileContext(nc) as tc:
        import contextlib

        with contextlib.ExitStack() as ctx:
            cpool = ctx.enter_context(tc.tile_pool(name="consts", bufs=1))
            apool = ctx.enter_context(tc.tile_pool(
                name="stageA", bufs=int(os.environ.get("ABUFS", "1"))))
            gpool = ctx.enter_context(tc.tile_pool(name="gather", bufs=int(os.environ.get("GBUFS", "4"))))
            gspool = ctx.enter_context(tc.tile_pool(name="gsum", bufs=int(os.environ.get("GSBUFS", "3"))))
            stpool = ctx.enter_context(tc.tile_pool(name="stmaj", bufs=int(os.environ.get("SBUFS", "4"))))
            opool = ctx.enter_context(tc.tile_pool(name="outsb", bufs=2))
            ptpool = ctx.enter_context(
                tc.tile_pool(name="psumT", bufs=2, space="PSUM")
            )
            pcpool = ctx.enter_context(
                tc.tile_pool(name="psumC", bufs=1, space="PSUM")
            )

            # ---- constants -------------------------------------------------
            xx = cpool.tile([128, 1], F32)
            nc.sync.dma_start(xx[:], xxd[:, :])
            yy = cpool.tile([128, SLOTS], F32)
            nc.sync.dma_start(yy[:], yyd[:, :])
            ident = cpool.tile([128, 128], F16)
            nc.sync.dma_start(ident[:], idd[:, :])
            wpt = cpool.tile([128, KK * OC], F16)
            nc.sync.dma_start(wpt[:], wp[:, :])
            bias = cpool.tile([OC, 1], F32)
            nc.sync.dma_start(bias[:], bsd[:, :])

            def body(_iv=None):
                if "gonly" in abl:
                    idxw = apool.tile([128, KK, SLOTS * 8], I16, name="idxw")
                    nc.gpsimd.iota(
                        idxw[:].rearrange("p a b -> p (a b)"),
                        pattern=[[3, KK * SLOTS * 8]],
                        base=0,
                        channel_multiplier=0,
                    )
                    xsrc = bass.AP(xp.tensor, 0, [[128, xrows - 1], [1, 256]])
                    regs = {6: nc.gpsimd.to_reg(768), 4: nc.gpsimd.to_reg(512)}
                    gi = 0
                    for ch in range(NCHUNK):
                        for t in range(KK):
                            g = gpool.tile([128, CSLOT, 4, 64], F16, name="g")
                            for s0, ns in ((0, 6), (6, 6), (12, 4)):
                                nc.gpsimd.dma_gather(
                                    g[:, s0 : s0 + ns, :, :].rearrange(
                                        "p s a c -> p s (a c)"
                                    ),
                                    xsrc,
                                    idxw[:, t, 128 * ch + 8 * s0 : 128 * ch + 8 * (s0 + ns)],
                                    num_idxs=ns * 128,
                                    num_idxs_reg=regs[ns],
                                    elem_size=256,
                                    elem_step=128,
                                    single_packet=False,
                                    queue_num=gi % 4,
                                )
                                gi += 1
                    return
                if "gonly2" in abl:
                    w4x, idxw = stage_a()
                    if "iotaidx" in abl:
                        idxw = apool.tile([128, KK, SLOTS * 8], I16, name="idxw2")
                        nc.gpsimd.iota(
                            idxw[:].rearrange("p a b -> p (a b)"),
                            pattern=[[3, KK * SLOTS * 8]],
                            base=0,
                            channel_multiplier=0,
                        )
                    xsrc = bass.AP(xp.tensor, 0, [[128, xrows - 1], [1, 256]])
                    regs = {6: nc.gpsimd.to_reg(768), 4: nc.gpsimd.to_reg(512)}
                    gi = 0
                    for ch in range(NCHUNK):
                        for t in range(KK):
                            g = gpool.tile([128, CSLOT, 4, 64], F16, name="g")
                            for s0, ns in ((0, 6), (6, 6), (12, 4)):
                                nc.gpsimd.dma_gather(
                                    g[:, s0 : s0 + ns, :, :].rearrange(
                                        "p s a c -> p s (a c)"
                                    ),
                                    xsrc,
                                    idxw[:, t, 128 * ch + 8 * s0 : 128 * ch + 8 * (s0 + ns)],
                                    num_idxs=ns * 128,
                                    num_idxs_reg=regs[ns],
                                    elem_size=256,
                                    elem_step=128,
                                    single_packet=False,
                                    queue_num=gi % int(os.environ.get("NQ", "4")),
                                )
                                gi += 1
                    return
                if "noa" in abl:
                    # gather-only isolation: iota indices, no stage A
                    idxw = apool.tile([128, KK, SLOTS * 8], I16, name="idxw")
                    nc.gpsimd.iota(
                        idxw[:].rearrange("p a b -> p (a b)"),
                        pattern=[[3, KK * SLOTS * 8]],
                        base=0,
                        channel_multiplier=0,
                    )
                    w4 = None
                else:
                    w4, idxw = stage_a()
                main_loops(w4, idxw)

            def stage_a():
                # ---- stage A: indices + weights, pipelined by tap-group ---
                # layout [128 part = pixel%128 (img col), slot = pixel//128
                # (img row), tap]. Computed in groups of 3 taps so the first
                # gathers can issue while the rest of stage A still runs.
                OFF = apool.tile([128, SLOTS, 2 * KK], F32, name="OFF")
                # host supplies offs pre-transposed to [p, (s c)] so this is a
                # contiguous 4.6KB-per-partition DMA (128 descriptors), not
                # the 8192x72B descriptor storm of a strided rearrange.
                nc.sync.dma_start(
                    OFF[:].rearrange("p s c -> p (s c)"), offs[:, :]
                )

                # w4d: corner weights stored with NDUP real duplicates along
                # the last dim so the g-multiply view has a packed last dim
                # ([1,NDUP]) -> DVE fast mode; measured on HW: 8 real copies
                # ([0,8],[1,8] view) runs 2.6x faster than the stride-0
                # broadcast ([0,64]).
                w4d = apool.tile([128, KK, SLOTS, 4, NDUP], F16, name="w4d")
                idx16 = apool.tile([128, KK, SLOTS], I16, name="idx16")
                idxw = apool.tile([128, KK, SLOTS * 8], I16, name="idxw")
                idxwv = idxw[:].rearrange("p t (s k) -> p t s k", k=8)
                tg = int(os.environ.get("TGRP", "3"))
                for t0 in range(0, KK, tg):
                    stage_a_group(OFF, w4d, idx16, idxw, idxwv, t0,
                                  min(tg, KK - t0))
                return w4d, idxw

            def stage_a_group(OFF, w4d, idx16, idxw, idxwv, t0, nt):
                offx = OFF[:, :, 2 * t0 : 2 * (t0 + nt) : 2]   # [128, 64, nt]
                offy = OFF[:, :, 2 * t0 + 1 : 2 * (t0 + nt) : 2]

                shp = [128, SLOTS, nt]

                def atile(name):
                    return apool.tile(shp, F32, name=name)

                vec = nc.vector
                px = atile("px")
                vec.tensor_scalar(px[:], offx, xx[:, 0:1], None, Alu.add)
                py = atile("py")
                yyb = yy[:, :].unsqueeze(2).broadcast_to(shp)
                vec.tensor_tensor(py[:], offy, yyb, Alu.add)
                # floor via int cast: F = round-ish(v) - (round-ish(v) > v)
                # exact for truncate or round-to-nearest cast semantics.
                casti = apool.tile(shp, mybir.dt.int32, name="casti")
                rnd = atile("rnd")
                tn = atile("tn")

                def floor_into(dst, v):
                    # dst <- floor(v)
                    vec.tensor_copy(casti[:], v[:])
                    vec.tensor_copy(rnd[:], casti[:])
                    vec.tensor_tensor(tn[:], rnd[:], v[:], Alu.is_gt)
                    vec.scalar_tensor_tensor(
                        dst[:], tn[:], -1.0, rnd[:], Alu.mult, Alu.add
                    )

                x0 = atile("x0")
                floor_into(x0, px)
                fx = atile("fx")
                vec.tensor_tensor(fx[:], px[:], x0[:], Alu.subtract)
                y0 = atile("y0")
                floor_into(y0, py)
                fy = atile("fy")
                vec.tensor_tensor(fy[:], py[:], y0[:], Alu.subtract)
                xc = atile("xc")
                clampmax = 130.0 if pad2 else 126.0
                vec.tensor_scalar(xc[:], x0[:], 0.0, clampmax, Alu.max, Alu.min)
                g0 = atile("g0")
                vec.tensor_scalar(g0[:], y0[:], 0.0, clampmax, Alu.max, Alu.min)
                # idx = g0*WP + xc (row-pair-per-row interleaved layout)
                idxf = atile("idxf")
                vec.tensor_scalar(
                    idxf[:], g0[:], float(WP if pad2 else 128), None, Alu.mult
                )
                vec.tensor_tensor(idxf[:], idxf[:], xc[:], Alu.add)
                vec.tensor_copy(
                    idx16[:, t0 : t0 + nt, :].rearrange("p t s -> p s t"),
                    idxf[:],
                )

                # weights. wx1=fx, wx0=1-fx
                wx0 = atile("wx0")
                vec.tensor_scalar(wx0[:], fx[:], -1.0, 1.0, Alu.mult, Alu.add)
                wy0 = atile("wy0")
                vec.tensor_scalar(wy0[:], fy[:], -1.0, 1.0, Alu.mult, Alu.add)

                if pad2:
                    # zero-framed layout: clamped fetches read zeros, so the
                    # corner weights are the plain bilinear products.
                    def wrt4p(n, wc, wr):
                        dst = w4d[:, t0 : t0 + nt, :, n, :].rearrange(
                            "p t s k -> p s t k"
                        )
                        shp2 = [128, SLOTS, nt, NDUP]
                        vec.tensor_tensor(
                            dst,
                            wc[:].unsqueeze(3).broadcast_to(shp2),
                            wr[:].unsqueeze(3).broadcast_to(shp2),
                            Alu.mult,
                        )

                    wrt4p(0, wx0, wy0)
                    wrt4p(1, wx0, fy)
                    wrt4p(2, fx, wy0)
                    wrt4p(3, fx, fy)
                    idx_shuffle(idx16, idxw, idxwv, t0, nt)
                    return

                dx = atile("dx")
                vec.tensor_tensor(dx[:], xc[:], x0[:], Alu.subtract)
                dy = atile("dy")
                vec.tensor_tensor(dy[:], g0[:], y0[:], Alu.subtract)
                e0 = atile("e0")
                vec.tensor_scalar(e0[:], dx[:], 0.0, None, Alu.is_equal)
                e1 = atile("e1")
                vec.tensor_scalar(e1[:], dx[:], 1.0, None, Alu.is_equal)
                em = atile("em")
                vec.tensor_scalar(em[:], dx[:], -1.0, None, Alu.is_equal)
                # wcL = wx0*e0 + fx*e1 ; wcR = wx0*em + fx*e0
                wcl = atile("wcl")
                vec.tensor_tensor(wcl[:], wx0[:], e0[:], Alu.mult)
                vec.tensor_tensor(e1[:], fx[:], e1[:], Alu.mult)
                vec.tensor_tensor(wcl[:], wcl[:], e1[:], Alu.add)
                wcr = atile("wcr")
                vec.tensor_tensor(wcr[:], wx0[:], em[:], Alu.mult)
                vec.tensor_tensor(e0[:], fx[:], e0[:], Alu.mult)
                vec.tensor_tensor(wcr[:], wcr[:], e0[:], Alu.add)
                # rows
                r0 = atile("r0")
                vec.tensor_scalar(r0[:], dy[:], 0.0, None, Alu.is_equal)
                r1 = atile("r1")
                vec.tensor_scalar(r1[:], dy[:], 1.0, None, Alu.is_equal)
                rm = atile("rm")
                vec.tensor_scalar(rm[:], dy[:], -1.0, None, Alu.is_equal)
                wrt = atile("wrt")
                vec.tensor_tensor(wrt[:], wy0[:], r0[:], Alu.mult)
                vec.tensor_tensor(r1[:], fy[:], r1[:], Alu.mult)
                vec.tensor_tensor(wrt[:], wrt[:], r1[:], Alu.add)
                wrb = atile("wrb")
                vec.tensor_tensor(wrb[:], wy0[:], rm[:], Alu.mult)
                vec.tensor_tensor(r0[:], fy[:], r0[:], Alu.mult)
                vec.tensor_tensor(wrb[:], wrb[:], r0[:], Alu.add)
                # w4d [128, tap, slot, n, 2] with n = (col, row):
                # 0=(L,T) 1=(L,B) 2=(R,T) 3=(R,B); each product written to
                # both k-slots via a real stride-1 out dim (ins bcast k).
                def wrt4(n, wc, wr):
                    dst = w4d[:, t0 : t0 + nt, :, n, :].rearrange(
                        "p t s k -> p s t k"
                    )
                    shp2 = [128, SLOTS, nt, NDUP]
                    vec.tensor_tensor(
                        dst,
                        wc[:].unsqueeze(3).broadcast_to(shp2),
                        wr[:].unsqueeze(3).broadcast_to(shp2),
                        Alu.mult,
                    )

                wrt4(0, wcl, wrt)
                wrt4(1, wcl, wrb)
                wrt4(2, wcr, wrt)
                wrt4(3, wcr, wrb)
                idx_shuffle(idx16, idxw, idxwv, t0, nt)

            def idx_shuffle(idx16, idxw, idxwv, t0, nt):
                # ---- idx shuffle into SWDGE wrapped-16 layout --------------
                # IDXW[q, t, s*8 + k] = idx16[16k+q, s, t]; replicated to all
                # eight 16-partition blocks.
                for k in range(8):
                    nc.sync.dma_start(
                        idxwv[0:16, t0 : t0 + nt, :, k].squeeze(),
                        idx16[16 * k : 16 * (k + 1), t0 : t0 + nt, :],
                    )
                # log-doubling replication: 16 -> 32 -> 64 -> 128 partitions
                span = 16
                while span < 128:
                    nc.sync.dma_start(
                        idxw[span : 2 * span, t0 : t0 + nt, :],
                        idxw[0:span, t0 : t0 + nt, :],
                    )
                    span *= 2

            def main_loops(w4d, idxw):
                vec = nc.vector
                # gather source view: row r -> 256 contiguous floats starting
                # at r*128 (overlapping windows)
                xsrc = bass.AP(xp.tensor, 0, [[128, xrows - 1], [1, 256]])  # fp16 rows
                gidx = [0]
                regs = {16: nc.gpsimd.to_reg(2048), 8: nc.gpsimd.to_reg(1024),
                        6: nc.gpsimd.to_reg(768), 4: nc.gpsimd.to_reg(512),
                        2: nc.gpsimd.to_reg(256)}
                # NOTE: PRESUM=pe (accumulating is_transpose matmuls) gives
                # WRONG results on HW -- transpose matmuls don't accumulate
                # in PSUM; the second transpose overwrites. Keep "dve".
                presum = os.environ.get("PRESUM", "off")

                # ---- main loop --------------------------------------------
                for ch in range(NCHUNK):
                    for t in range(KK):
                        g = gpool.tile([128, CSLOT, 4, 64], F16, name="g")
                        if "nogather" in abl and ch + t == 0:
                            nc.vector.memset(g[:], 0.25)
                        # sub-gathers sized to the SWDGE ring (DDS/16 descs)
                        subenv = os.environ.get("SUBS", "664")
                        if subenv == "16":
                            subs = ((0, 16),)
                        elif subenv == "88":
                            subs = ((0, 8), (8, 8))
                        elif subenv == "664":
                            subs = ((0, 6), (6, 6), (12, 4))
                        elif subenv == "44":
                            subs = ((0, 4), (4, 4), (8, 4), (12, 4))
                        else:
                            subs = ((0, 2), (2, 2), (4, 2), (6, 2),
                                    (8, 2), (10, 2), (12, 2), (14, 2))
                        if "nogather" in abl:
                            subs = ()
                        for s0, ns in subs:
                            nidx = ns * 128
                            qn = gidx[0] % int(os.environ.get("NQ", "4"))
                            nc.gpsimd.dma_gather(
                                g[:, s0 : s0 + ns, :, :].rearrange(
                                    "p s a c -> p s (a c)"
                                ),
                                xsrc,
                                idxw[
                                    :, t,
                                    128 * ch + 8 * s0 : 128 * ch + 8 * (s0 + ns),
                                ],
                                num_idxs=nidx,
                                num_idxs_reg=regs[ns],
                                elem_size=256,
                                elem_step=128,
                                single_packet=False,
                                queue_num=qn,
                            )
                            gidx[0] += 1
                        # combine: g *= corner weight (dup-pair view keeps the
                        # last AP dim packed -> DVE 2x_1p fast mode); the
                        # col-corner ADD is presummed so transposes + ACT
                        # copies + conv matmuls all halve; the remaining
                        # row-corner add is folded into the conv contraction.
                        if "nodve" not in abl:
                            if wmul in ("dup", "dup4", "dup8"):
                                gv = g[:].rearrange(
                                    "p s a (x k) -> p (s a) x k", k=NDUP
                                )
                                wv = (
                                    w4d[:, t, ch * CSLOT : (ch + 1) * CSLOT, :, :]
                                    .rearrange("p s n k -> p (s n) k")
                                    .unsqueeze(2)
                                    .broadcast_to(
                                        [128, 4 * CSLOT, 64 // NDUP, NDUP]
                                    )
                                )
                                vec.tensor_tensor(gv, gv, wv, Alu.mult)
                            else:
                                wsl = (
                                    w4d[:, t, ch * CSLOT : (ch + 1) * CSLOT, :, 0:1]
                                    .broadcast_to([128, CSLOT, 4, 64])
                                )
                                vec.tensor_tensor(g[:], g[:], wsl, Alu.mult)
                        if "nope" not in abl:
                            if presum == "dve":
                                # DVE presum of the col-corners: halves PE/ACT
                                # work but adds a DVE op inside the gather
                                # phase (DVE time adds ~1:1 there on HW).
                                stm = stpool.tile([128, CPIX], F16, name="stm")
                                pt = ptpool.tile([128, CPIX], F16, name="pt")
                                gs = gspool.tile(
                                    [128, CSLOT, 128], F16, name="gs"
                                )
                                vec.tensor_tensor(
                                    gs[:],
                                    g[:, :, 0:2, :].rearrange(
                                        "p s a c -> p s (a c)"
                                    ),
                                    g[:, :, 2:4, :].rearrange(
                                        "p s a c -> p s (a c)"
                                    ),
                                    Alu.add,
                                )
                                for slot in range(CSLOT):
                                    nc.tensor.matmul(
                                        pt[:, 128 * slot : 128 * (slot + 1)],
                                        gs[:, slot, :],
                                        ident[:],
                                        is_transpose=True,
                                    )
                                nc.scalar.activation(stm[:], pt[:], Act.Copy)
                                if t == 0:
                                    pc = pcpool.tile([OC, CPIX], F32, name="pc")
                                for nb in range(CPIX // 512):
                                    nc.tensor.matmul(
                                        pc[:, 512 * nb : 512 * (nb + 1)],
                                        wpt[:, OC * t : OC * (t + 1)],
                                        stm[:, 512 * nb : 512 * (nb + 1)],
                                        start=(t == 0),
                                        stop=(t == KK - 1),
                                    )
                            else:
                                # presum "off": all 4 corner adds fold into
                                # the conv contraction (K=256 over 2 passes);
                                # PE/ACT do 2x work but run in the shadow of
                                # the gather+DVE phase.
                                stm = stpool.tile(
                                    [128, 2, CPIX], F16, name="stm"
                                )
                                for hb in range(2):
                                    pt = ptpool.tile(
                                        [128, CPIX], F16, name="pt"
                                    )
                                    for slot in range(CSLOT):
                                        nc.tensor.matmul(
                                            pt[:, 128 * slot : 128 * (slot + 1)],
                                            g[:, slot, 2 * hb : 2 * hb + 2, :],
                                            ident[:],
                                            is_transpose=True,
                                        )
                                    nc.scalar.activation(
                                        stm[:, hb, :], pt[:], Act.Copy
                                    )
                                if t == 0:
                                    pc = pcpool.tile([OC, CPIX], F32, name="pc")
                                for hb in range(2):
                                    for nb in range(CPIX // 512):
                                        nc.tensor.matmul(
                                            pc[:, 512 * nb : 512 * (nb + 1)],
                                            wpt[:, OC * t : OC * (t + 1)],
                                            stm[:, hb, 512 * nb : 512 * (nb + 1)],
                                            start=(t == 0 and hb == 0),
                                            stop=(t == KK - 1 and hb == 1),
                                        )
                    if "nope" in abl:
                        nc.gpsimd.dma_start(
                            out[0:64, CPIX * ch : CPIX * ch + 1024],
                            g[0:64, :, :, :].rearrange("p a b c -> p (a b c)")[
                                :, 0:1024
                            ],
                        )
                    else:
                        osb = opool.tile([OC, CPIX], F32, name="osb")
                        nc.scalar.activation(
                            osb[:], pc[:], Act.Identity, bias=bias[:, 0:1]
                        )
                        nc.sync.dma_start(
                            out[:, CPIX * ch : CPIX * (ch + 1)], osb[:]
                        )

            if loop_n > 0:
                with tc.For_i(0, loop_n, 1):
                    body()
            elif loop_n < 0:
                for _ in range(-loop_n):
                    body()
            else:
                body()
            if "gonly" in abl or "gonly2" in abl:
                dummy = cpool.tile([OC, 64], F32)
                nc.vector.memset(dummy[:], 1.0)
                nc.sync.dma_start(out[:, 0:64], dummy[:])

    nc.compile()
    return nc


def prep_core_inputs(x, offset, weight, bias, core):
    """Host-side shard/layout prep for one core. Pure layout, no math on
    tensor values (beyond the reference-mandated reshape semantics)."""
    s, half = core // 2, core % 2
    pad2 = os.environ.get("PAD", "2") == "2"
    xr = np.ascontiguousarray(x[s].transpose(1, 2, 0))          # [H, W, C]
    if pad2:
        # zero-framed image (+2 texels each side); pair-row (g0, j) holds
        # padded rows g0, g0+1 at padded col j.
        xf = np.zeros((H + 4, WP, C), np.float32)
        xf[2 : H + 2, 2 : W + 2] = xr
        rows = np.stack([xf[0:GP], xf[1 : GP + 1]], 1)   # [GP, 2, WP, C]
        xpair = np.concatenate(
            [rows.transpose(0, 2, 1, 3).reshape(GP * WP, 128),
             np.zeros((1, 128), np.float32)], 0)
    else:
        # interleaved row-pair NHWC: xpair[par*PIX + pr*128 + j] =
        #   [x[2pr+par, j, :], x[2pr+par+1, j, :]]
        xpad = np.concatenate([xr, np.zeros((1, W, C), np.float32)], 0)  # [129,W,C]
        rows = np.stack([xpad[0:128], xpad[1:129]], 1)           # [128, 2, W, C]
        # xpair row (g0, j) = [x[g0, j, :], x[g0+1, j, :]]
        xpair = np.concatenate(
            [rows.transpose(0, 2, 1, 3).reshape(2 * PIX, 128),
             np.zeros((1, 128), np.float32)], 0)

    # offsets for this half: [p=col, (slot, chan)] so the device load is
    # one contiguous row per partition
    off = np.ascontiguousarray(
        offset[s, :, 64 * half : 64 * half + HALF, :]
        .transpose(2, 1, 0)            # [W=128, HALF slots, 18]
        .reshape(128, SLOTS * 2 * KK)
    )
    # channel c=2t is x-offset, 2t+1 is y-offset (reference reshape
    # [kk,2,h,w]: x = off[:,:,0], y = off[:,:,1] -> channel t*2+0 / t*2+1)

    # tap-pair weight slabs [128, 5*64]: rows sub*64+c, cols pair*64+o
    wfull = weight.reshape(OC, C, KK)
    wpair = np.zeros((128, KK * OC), np.float32)
    for t in range(KK):
        wt = wfull[:, :, t].T          # [c, o]
        wpair[0:64, t * OC : (t + 1) * OC] = wt
        wpair[64:128, t * OC : (t + 1) * OC] = wt

    shift = 2.0 if pad2 else 0.0
    yy = np.broadcast_to(
        (np.arange(SLOTS, dtype=np.float32) + 64 * half + shift)[None, :],
        (128, SLOTS),
    ).copy()
    xxc = (np.arange(128, dtype=np.float32) + shift).reshape(128, 1).copy()
    return {
        "xpair": np.ascontiguousarray(xpair).astype(np.float16),
        "offs": off.astype(np.float32),
        "wpair": wpair.astype(np.float16),
        "yy": yy,
        "xx": xxc,
        "ident": np.eye(128, dtype=np.float16),
        "bias": bias.reshape(OC, 1).astype(np.float32),
    }


_CACHE = {}


def kernel(x, offset, weight, bias):
    x = np.asarray(x, np.float32)
    offset = np.asarray(offset, np.float32)
    weight = np.asarray(weight, np.float32)
    bias = np.asarray(bias, np.float32)
    if "nc" not in _CACHE:
        _CACHE["nc"] = build_program()
    nc = _CACHE["nc"]
    in_maps = [prep_core_inputs(x, offset, weight, bias, c) for c in range(8)]
    res = run_bass_kernel_spmd(nc, in_maps, core_ids=list(range(8)))
    outf = np.empty((B, OC, H, W), np.float32)
    for c in range(8):
        s, half = c // 2, c % 2
        outf[s, :, 64 * half : 64 * half + HALF, :] = res.results[c][
            "out"
        ].reshape(OC, HALF, W)
    return outf



# revision 27
# speedup vs baseline: 1.1239x; 1.1239x over previous
"""Deformable Conv2d Lite (K=3) on 8 Trainium2 NeuronCores.

Sharding: data-parallel over batch x image-half. Core n handles sample n//2,
image rows [64*(n%2), 64*(n%2)+64). Weight replicated.

Device pipeline per core (rel err ~6e-4):
  1. DVE stage A: from raw offsets compute, per (tap, pixel), a gather index
     into a row-pair-interleaved fp16 NHWC layout of a ZERO-FRAMED (+2 texel
     border) image (pair-row r of xpair holds padded rows r, r+1 at one col;
     idx = clamp(y0+2, 0, 130)*132 + clamp(x0+2, 0, 130)); because clamped
     out-of-range fetches land on zero texels, the 4 bilinear corner weights
     are the plain products wx{0,1}*wy{0,1} -- no edge eq-masking (floor is
     int-cast + is_gt since mod is not a valid HW ALU op). Weights land in
     w4d with 8 REAL duplicates along the last dim (see step 3).
  2. SWDGE dma_gather (fp16): one 512B descriptor per (tap, pixel) fetches
     the full 2x2 x 64ch bilinear patch from DRAM; 108 calls of <=768
     indices (descriptor-ring capacity) round-robined over 4 SWDGE queues.
  3. DVE: in-place weight multiply with the weight operand viewed as
     [.., (slot corner), 8, [1,8]] over the 8 real duplicates: a packed
     last dim hits the DVE fast path -- measured 1.26us vs 3.3us/op for the
     stride-0 [0,64] broadcast. ALL four corner ADDs are folded into the
     conv matmul contraction (K=256 over 2 passes): on this HW, DVE
     elementwise time adds ~1:1 on top of the gather phase (SBUF port
     contention with Pool descriptor-gen + inbound gather writes), while
     PE/ACT run entirely in its shadow (~245us headroom measured via
     ablations), so work is pushed off DVE wherever possible. (Tried and
     rejected: accumulating is_transpose matmuls -- PE transposes do NOT
     accumulate in PSUM, they overwrite; DVE col-corner presum -- correct
     but adds DVE time for PE/ACT savings that were already free.)
  4. PE: fp16 transposes of the weighted patches to channel-major
     (K = 2 corners x 64ch = 128 per tap-half), then conv matmuls with
     per-tap weight slabs (W_t.T stacked twice) accumulating f32 in PSUM
     over all 9 taps x 2 halves.
  5. ACT: PSUM->SBUF copies of the transposed tiles, and final bias-add on
     the conv PSUM; DMA out.
"""

import sys

for _p in ("/opt/trn_rl_repo",):
    if _p not in sys.path:
        sys.path.insert(0, _p)

import numpy as np

import concourse.bass as bass
import concourse.tile as tile
from concourse import bacc, mybir
from concourse.bass_utils import run_bass_kernel_spmd

F32 = mybir.dt.float32
F16 = mybir.dt.float16
I16 = mybir.dt.int16
Alu = mybir.AluOpType
Act = mybir.ActivationFunctionType

B, C, H, W = 4, 64, 128, 128
OC, KK = 64, 9
HALF = H // 2            # rows per core
PIX = HALF * W           # 8192 pixels per core
NCHUNK = 4
CPIX = PIX // NCHUNK     # 2048 pixels per chunk
CSLOT = CPIX // 128      # 16 slots per chunk
SLOTS = PIX // 128       # 64
NPAIR = 5                # ceil(9/2) tap pairs

XROWS = 2 * PIX + 1      # interleaved pair-row count incl. pad (PAD=0 mode)
# PAD=2 mode: image zero-framed with 2 texels per side -> clamped OOB
# fetches read zeros, so bilinear weights need no edge masking.
WP = W + 4               # padded cols
GP = H + 3               # pair-row groups (g0 in 0..130)
XROWSP = GP * WP + 1     # 17293


import os


def build_program(loop_n: int = 0, ablate: str = ""):
    """Build the per-core Bass program. loop_n>0 wraps the body in a device
    For_i loop (for wall-clock timing); loop_n==0 emits the plain body.
    ablate: comma-set of {nogather, nodve, nope} for perf bisection."""
    abl = set(ablate.split(",")) if ablate else set()
    import os
    pad2 = os.environ.get("PAD", "2") == "2"
    wmul = os.environ.get("WMUL", "dup8")
    NDUP = {"dup8": 8, "dup4": 4, "dup": 2, "bcast": 2}[wmul]
    xrows = XROWSP if pad2 else XROWS
    nc = bacc.Bacc("TRN2", target_bir_lowering=False, debug=False, num_devices=8,
                   num_swdge_queues=int(os.environ.get("NSQ", "4")),
                   dynamic_dma_scratch_size=int(os.environ.get("DDS", "16384")))

    xp = nc.dram_tensor("xpair", [xrows, 128], F16, kind="ExternalInput").ap()
    offs = nc.dram_tensor("offs", [128, SLOTS * 2 * KK], F32, kind="ExternalInput").ap()
    wp = nc.dram_tensor("wpair", [128, KK * OC], F16, kind="ExternalInput").ap()
    yyd = nc.dram_tensor("yy", [128, SLOTS], F32, kind="ExternalInput").ap()
    xxd = nc.dram_tensor("xx", [128, 1], F32, kind="ExternalInput").ap()
    idd = nc.dram_tensor("ident", [128, 128], F16, kind="ExternalInput").ap()
    bsd = nc.dram_tensor("bias", [OC, 1], F32, kind="ExternalInput").ap()
    out = nc.dram_tensor("out", [OC, PIX], F32, kind="ExternalOutput").ap()

    with tile.TileContext(nc) as tc:
        import contextlib

        with contextlib.ExitStack() as ctx:
            cpool = ctx.enter_context(tc.tile_pool(name="consts", bufs=1))
            apool = ctx.enter_context(tc.tile_pool(
                name="stageA", bufs=int(os.environ.get("ABUFS", "1"))))
            gpool = ctx.enter_context(tc.tile_pool(name="gather", bufs=int(os.environ.get("GBUFS", "4"))))
            gspool = ctx.enter_context(tc.tile_pool(name="gsum", bufs=int(os.environ.get("GSBUFS", "3"))))
            stpool = ctx.enter_context(tc.tile_pool(name="stmaj", bufs=int(os.environ.get("SBUFS", "4"))))
            opool = ctx.enter_context(tc.tile_pool(name="outsb", bufs=2))
            ptpool = ctx.enter_context(
                tc.tile_pool(name="psumT", bufs=2, space="PSUM")
            )
            pcpool = ctx.enter_context(
                tc.tile_pool(name="psumC", bufs=1, space="PSUM")
            )

            # ---- constants -------------------------------------------------
            xx = cpool.tile([128, 1], F32)
            nc.sync.dma_start(xx[:], xxd[:, :])
            yy = cpool.tile([128, SLOTS], F32)
            nc.sync.dma_start(yy[:], yyd[:, :])
            ident = cpool.tile([128, 128], F16)
            nc.sync.dma_start(ident[:], idd[:, :])
            wpt = cpool.tile([128, KK * OC], F16)
            nc.sync.dma_start(wpt[:], wp[:, :])
            bias = cpool.tile([OC, 1], F32)
            nc.sync.dma_start(bias[:], bsd[:, :])

            def body(_iv=None):
                if "gonly" in abl:
                    idxw = apool.tile([128, KK, SLOTS * 8], I16, name="idxw")
                    nc.gpsimd.iota(
                        idxw[:].rearrange("p a b -> p (a b)"),
                        pattern=[[3, KK * SLOTS * 8]],
                        base=0,
                        channel_multiplier=0,
                    )
                    xsrc = bass.AP(xp.tensor, 0, [[128, xrows - 1], [1, 256]])
                    regs = {6: nc.gpsimd.to_reg(768), 4: nc.gpsimd.to_reg(512)}
                    gi = 0
                    for ch in range(NCHUNK):
                        for t in range(KK):
                            g = gpool.tile([128, CSLOT, 4, 64], F16, name="g")
                            for s0, ns in ((0, 6), (6, 6), (12, 4)):
                                nc.gpsimd.dma_gather(
                                    g[:, s0 : s0 + ns, :, :].rearrange(
                                        "p s a c -> p s (a c)"
                                    ),
                                    xsrc,
                                    idxw[:, t, 128 * ch + 8 * s0 : 128 * ch + 8 * (s0 + ns)],
                                    num_idxs=ns * 128,
                                    num_idxs_reg=regs[ns],
                                    elem_size=256,
                                    elem_step=128,
                                    single_packet=False,
                                    queue_num=gi % 4,
                                )
                                gi += 1
                    return
                if "gonly2" in abl:
                    w4x, idxw = stage_a()
                    if "iotaidx" in abl:
                        idxw = apool.tile([128, KK, SLOTS * 8], I16, name="idxw2")
                        nc.gpsimd.iota(
                            idxw[:].rearrange("p a b -> p (a b)"),
                            pattern=[[3, KK * SLOTS * 8]],
                            base=0,
                            channel_multiplier=0,
                        )
                    xsrc = bass.AP(xp.tensor, 0, [[128, xrows - 1], [1, 256]])
                    regs = {6: nc.gpsimd.to_reg(768), 4: nc.gpsimd.to_reg(512)}
                    gi = 0
                    for ch in range(NCHUNK):
                        for t in range(KK):
                            g = gpool.tile([128, CSLOT, 4, 64], F16, name="g")
                            for s0, ns in ((0, 6), (6, 6), (12, 4)):
                                nc.gpsimd.dma_gather(
                                    g[:, s0 : s0 + ns, :, :].rearrange(
                                        "p s a c -> p s (a c)"
                                    ),
                                    xsrc,
                                    idxw[:, t, 128 * ch + 8 * s0 : 128 * ch + 8 * (s0 + ns)],
                                    num_idxs=ns * 128,
                                    num_idxs_reg=regs[ns],
                                    elem_size=256,
                                    elem_step=128,
                                    single_packet=False,
                                    queue_num=gi % int(os.environ.get("NQ", "4")),
                                )
                                gi += 1
                    return
                if "noa" in abl:
                    # gather-only isolation: iota indices, no stage A
                    idxw = apool.tile([128, KK, SLOTS * 8], I16, name="idxw")
                    nc.gpsimd.iota(
                        idxw[:].rearrange("p a b -> p (a b)"),
                        pattern=[[3, KK * SLOTS * 8]],
                        base=0,
                        channel_multiplier=0,
                    )
                    w4 = None
                else:
                    w4, idxw = stage_a()
                main_loops(w4, idxw)

            def stage_a():
                # ---- stage A: indices + weights, pipelined by tap-group ---
                # layout [128 part = pixel%128 (img col), slot = pixel//128
                # (img row), tap]. Computed in groups of 3 taps so the first
                # gathers can issue while the rest of stage A still runs.
                OFF = apool.tile([128, SLOTS, 2 * KK], F32, name="OFF")
                # host supplies offs pre-transposed to [p, (s c)] so this is a
                # contiguous 4.6KB-per-partition DMA (128 descriptors), not
                # the 8192x72B descriptor storm of a strided rearrange.
                nc.sync.dma_start(
                    OFF[:].rearrange("p s c -> p (s c)"), offs[:, :]
                )

                # w4d: corner weights stored with NDUP real duplicates along
                # the last dim so the g-multiply view has a packed last dim
                # ([1,NDUP]) -> DVE fast mode; measured on HW: 8 real copies
                # ([0,8],[1,8] view) runs 2.6x faster than the stride-0
                # broadcast ([0,64]).
                w4d = apool.tile([128, KK, SLOTS, 4, NDUP], F16, name="w4d")
                idx16 = apool.tile([128, KK, SLOTS], I16, name="idx16")
                idxw = apool.tile([128, KK, SLOTS * 8], I16, name="idxw")
                idxwv = idxw[:].rearrange("p t (s k) -> p t s k", k=8)
                tg = int(os.environ.get("TGRP", "3"))
                for t0 in range(0, KK, tg):
                    stage_a_group(OFF, w4d, idx16, idxw, idxwv, t0,
                                  min(tg, KK - t0))
                return w4d, idxw

            def stage_a_group(OFF, w4d, idx16, idxw, idxwv, t0, nt):
                offx = OFF[:, :, 2 * t0 : 2 * (t0 + nt) : 2]   # [128, 64, nt]
                offy = OFF[:, :, 2 * t0 + 1 : 2 * (t0 + nt) : 2]

                shp = [128, SLOTS, nt]

                def atile(name):
                    return apool.tile(shp, F32, name=name)

                vec = nc.vector
                px = atile("px")
                vec.tensor_scalar(px[:], offx, xx[:, 0:1], None, Alu.add)
                py = atile("py")
                yyb = yy[:, :].unsqueeze(2).broadcast_to(shp)
                vec.tensor_tensor(py[:], offy, yyb, Alu.add)
                # floor via int cast: F = round-ish(v) - (round-ish(v) > v)
                # exact for truncate or round-to-nearest cast semantics.
                casti = apool.tile(shp, mybir.dt.int32, name="casti")
                rnd = atile("rnd")
                tn = atile("tn")

                def floor_into(dst, v):
                    # dst <- floor(v)
                    vec.tensor_copy(casti[:], v[:])
                    vec.tensor_copy(rnd[:], casti[:])
                    vec.tensor_tensor(tn[:], rnd[:], v[:], Alu.is_gt)
                    vec.scalar_tensor_tensor(
                        dst[:], tn[:], -1.0, rnd[:], Alu.mult, Alu.add
                    )

                x0 = atile("x0")
                floor_into(x0, px)
                fx = atile("fx")
                vec.tensor_tensor(fx[:], px[:], x0[:], Alu.subtract)
                y0 = atile("y0")
                floor_into(y0, py)
                fy = atile("fy")
                vec.tensor_tensor(fy[:], py[:], y0[:], Alu.subtract)
                xc = atile("xc")
                clampmax = 130.0 if pad2 else 126.0
                vec.tensor_scalar(xc[:], x0[:], 0.0, clampmax, Alu.max, Alu.min)
                g0 = atile("g0")
                vec.tensor_scalar(g0[:], y0[:], 0.0, clampmax, Alu.max, Alu.min)
                # idx = g0*WP + xc (row-pair-per-row interleaved layout)
                idxf = atile("idxf")
                vec.tensor_scalar(
                    idxf[:], g0[:], float(WP if pad2 else 128), None, Alu.mult
                )
                vec.tensor_tensor(idxf[:], idxf[:], xc[:], Alu.add)
                vec.tensor_copy(
                    idx16[:, t0 : t0 + nt, :].rearrange("p t s -> p s t"),
                    idxf[:],
                )

                # weights. wx1=fx, wx0=1-fx
                wx0 = atile("wx0")
                vec.tensor_scalar(wx0[:], fx[:], -1.0, 1.0, Alu.mult, Alu.add)
                wy0 = atile("wy0")
                vec.tensor_scalar(wy0[:], fy[:], -1.0, 1.0, Alu.mult, Alu.add)

                if pad2:
                    # zero-framed layout: clamped fetches read zeros, so the
                    # corner weights are the plain bilinear products.
                    def wrt4p(n, wc, wr):
                        dst = w4d[:, t0 : t0 + nt, :, n, :].rearrange(
                            "p t s k -> p s t k"
                        )
                        shp2 = [128, SLOTS, nt, NDUP]
                        vec.tensor_tensor(
                            dst,
                            wc[:].unsqueeze(3).broadcast_to(shp2),
                            wr[:].unsqueeze(3).broadcast_to(shp2),
                            Alu.mult,
                        )

                    wrt4p(0, wx0, wy0)
                    wrt4p(1, wx0, fy)
                    wrt4p(2, fx, wy0)
                    wrt4p(3, fx, fy)
                    idx_shuffle(idx16, idxw, idxwv, t0, nt)
                    return

                dx = atile("dx")
                vec.tensor_tensor(dx[:], xc[:], x0[:], Alu.subtract)
                dy = atile("dy")
                vec.tensor_tensor(dy[:], g0[:], y0[:], Alu.subtract)
                e0 = atile("e0")
                vec.tensor_scalar(e0[:], dx[:], 0.0, None, Alu.is_equal)
                e1 = atile("e1")
                vec.tensor_scalar(e1[:], dx[:], 1.0, None, Alu.is_equal)
                em = atile("em")
                vec.tensor_scalar(em[:], dx[:], -1.0, None, Alu.is_equal)
                # wcL = wx0*e0 + fx*e1 ; wcR = wx0*em + fx*e0
                wcl = atile("wcl")
                vec.tensor_tensor(wcl[:], wx0[:], e0[:], Alu.mult)
                vec.tensor_tensor(e1[:], fx[:], e1[:], Alu.mult)
                vec.tensor_tensor(wcl[:], wcl[:], e1[:], Alu.add)
                wcr = atile("wcr")
                vec.tensor_tensor(wcr[:], wx0[:], em[:], Alu.mult)
                vec.tensor_tensor(e0[:], fx[:], e0[:], Alu.mult)
                vec.tensor_tensor(wcr[:], wcr[:], e0[:], Alu.add)
                # rows
                r0 = atile("r0")
                vec.tensor_scalar(r0[:], dy[:], 0.0, None, Alu.is_equal)
                r1 = atile("r1")
                vec.tensor_scalar(r1[:], dy[:], 1.0, None, Alu.is_equal)
                rm = atile("rm")
                vec.tensor_scalar(rm[:], dy[:], -1.0, None, Alu.is_equal)
                wrt = atile("wrt")
                vec.tensor_tensor(wrt[:], wy0[:], r0[:], Alu.mult)
                vec.tensor_tensor(r1[:], fy[:], r1[:], Alu.mult)
                vec.tensor_tensor(wrt[:], wrt[:], r1[:], Alu.add)
                wrb = atile("wrb")
                vec.tensor_tensor(wrb[:], wy0[:], rm[:], Alu.mult)
                vec.tensor_tensor(r0[:], fy[:], r0[:], Alu.mult)
                vec.tensor_tensor(wrb[:], wrb[:], r0[:], Alu.add)
                # w4d [128, tap, slot, n, 2] with n = (col, row):
                # 0=(L,T) 1=(L,B) 2=(R,T) 3=(R,B); each product written to
                # both k-slots via a real stride-1 out dim (ins bcast k).
                def wrt4(n, wc, wr):
                    dst = w4d[:, t0 : t0 + nt, :, n, :].rearrange(
                        "p t s k -> p s t k"
                    )
                    shp2 = [128, SLOTS, nt, NDUP]
                    vec.tensor_tensor(
                        dst,
                        wc[:].unsqueeze(3).broadcast_to(shp2),
                        wr[:].unsqueeze(3).broadcast_to(shp2),
                        Alu.mult,
                    )

                wrt4(0, wcl, wrt)
                wrt4(1, wcl, wrb)
                wrt4(2, wcr, wrt)
                wrt4(3, wcr, wrb)
                idx_shuffle(idx16, idxw, idxwv, t0, nt)

            def idx_shuffle(idx16, idxw, idxwv, t0, nt):
                # ---- idx shuffle into SWDGE wrapped-16 layout --------------
                # IDXW[q, t, s*8 + k] = idx16[16k+q, s, t]; replicated to all
                # eight 16-partition blocks.
                for k in range(8):
                    nc.sync.dma_start(
                        idxwv[0:16, t0 : t0 + nt, :, k].squeeze(),
                        idx16[16 * k : 16 * (k + 1), t0 : t0 + nt, :],
                    )
                # log-doubling replication: 16 -> 32 -> 64 -> 128 partitions
                span = 16
                while span < 128:
                    nc.sync.dma_start(
                        idxw[span : 2 * span, t0 : t0 + nt, :],
                        idxw[0:span, t0 : t0 + nt, :],
                    )
                    span *= 2

            def main_loops(w4d, idxw):
                vec = nc.vector
                # gather source view: row r -> 256 contiguous floats starting
                # at r*128 (overlapping windows)
                xsrc = bass.AP(xp.tensor, 0, [[128, xrows - 1], [1, 256]])  # fp16 rows
                gidx = [0]
                regs = {16: nc.gpsimd.to_reg(2048), 8: nc.gpsimd.to_reg(1024),
                        6: nc.gpsimd.to_reg(768), 4: nc.gpsimd.to_reg(512),
                        2: nc.gpsimd.to_reg(256)}
                # NOTE: PRESUM=pe (accumulating is_transpose matmuls) gives
                # WRONG results on HW -- transpose matmuls don't accumulate
                # in PSUM; the second transpose overwrites. Keep "dve".
                presum = os.environ.get("PRESUM", "off")

                # ---- main loop --------------------------------------------
                for ch in range(NCHUNK):
                    for t in range(KK):
                        g = gpool.tile([128, CSLOT, 4, 64], F16, name="g")
                        if "nogather" in abl and ch + t == 0:
                            nc.vector.memset(g[:], 0.25)
                        # sub-gathers sized to the SWDGE ring (DDS/16 descs)
                        subenv = os.environ.get("SUBS", "664")
                        if subenv == "16":
                            subs = ((0, 16),)
                        elif subenv == "88":
                            subs = ((0, 8), (8, 8))
                        elif subenv == "664":
                            subs = ((0, 6), (6, 6), (12, 4))
                        elif subenv == "44":
                            subs = ((0, 4), (4, 4), (8, 4), (12, 4))
                        else:
                            subs = ((0, 2), (2, 2), (4, 2), (6, 2),
                                    (8, 2), (10, 2), (12, 2), (14, 2))
                        if "nogather" in abl:
                            subs = ()
                        for s0, ns in subs:
                            nidx = ns * 128
                            qn = gidx[0] % int(os.environ.get("NQ", "4"))
                            nc.gpsimd.dma_gather(
                                g[:, s0 : s0 + ns, :, :].rearrange(
                                    "p s a c -> p s (a c)"
                                ),
                                xsrc,
                                idxw[
                                    :, t,
                                    128 * ch + 8 * s0 : 128 * ch + 8 * (s0 + ns),
                                ],
                                num_idxs=nidx,
                                num_idxs_reg=regs[ns],
                                elem_size=256,
                                elem_step=128,
                                single_packet=False,
                                queue_num=qn,
                            )
                            gidx[0] += 1
                        # combine: g *= corner weight (dup-pair view keeps the
                        # last AP dim packed -> DVE 2x_1p fast mode); the
                        # col-corner ADD is presummed so transposes + ACT
                        # copies + conv matmuls all halve; the remaining
                        # row-corner add is folded into the conv contraction.
                        if "nodve" not in abl:
                            if wmul in ("dup", "dup4", "dup8"):
                                gv = g[:].rearrange(
                                    "p s a (x k) -> p (s a) x k", k=NDUP
                                )
                                wv = (
                                    w4d[:, t, ch * CSLOT : (ch + 1) * CSLOT, :, :]
                                    .rearrange("p s n k -> p (s n) k")
                                    .unsqueeze(2)
                                    .broadcast_to(
                                        [128, 4 * CSLOT, 64 // NDUP, NDUP]
                                    )
                                )
                                vec.tensor_tensor(gv, gv, wv, Alu.mult)
                            else:
                                wsl = (
                                    w4d[:, t, ch * CSLOT : (ch + 1) * CSLOT, :, 0:1]
                                    .broadcast_to([128, CSLOT, 4, 64])
                                )
                                vec.tensor_tensor(g[:], g[:], wsl, Alu.mult)
                        if "nope" not in abl:
                            if presum == "dve":
                                # DVE presum of the col-corners: halves PE/ACT
                                # work but adds a DVE op inside the gather
                                # phase (DVE time adds ~1:1 there on HW).
                                stm = stpool.tile([128, CPIX], F16, name="stm")
                                pt = ptpool.tile([128, CPIX], F16, name="pt")
                                gs = gspool.tile(
                                    [128, CSLOT, 128], F16, name="gs"
                                )
                                vec.tensor_tensor(
                                    gs[:],
                                    g[:, :, 0:2, :].rearrange(
                                        "p s a c -> p s (a c)"
                                    ),
                                    g[:, :, 2:4, :].rearrange(
                                        "p s a c -> p s (a c)"
                                    ),
                                    Alu.add,
                                )
                                for slot in range(CSLOT):
                                    nc.tensor.matmul(
                                        pt[:, 128 * slot : 128 * (slot + 1)],
                                        gs[:, slot, :],
                                        ident[:],
                                        is_transpose=True,
                                    )
                                nc.scalar.activation(stm[:], pt[:], Act.Copy)
                                if t == 0:
                                    pc = pcpool.tile([OC, CPIX], F32, name="pc")
                                for nb in range(CPIX // 512):
                                    nc.tensor.matmul(
                                        pc[:, 512 * nb : 512 * (nb + 1)],
                                        wpt[:, OC * t : OC * (t + 1)],
                                        stm[:, 512 * nb : 512 * (nb + 1)],
                                        start=(t == 0),
                                        stop=(t == KK - 1),
                                    )
                            else:
                                # presum "off": all 4 corner adds fold into
                                # the conv contraction (K=256 over 2 passes);
                                # PE/ACT do 2x work but run in the shadow of
                                # the gather+DVE phase.
                                stm = stpool.tile(
                                    [128, 2, CPIX], F16, name="stm"
                                )
                                for hb in range(2):
                                    pt = ptpool.tile(
                                        [128, CPIX], F16, name="pt"
                                    )
                                    for slot in range(CSLOT):
                                        nc.tensor.matmul(
                                            pt[:, 128 * slot : 128 * (slot + 1)],
                                            g[:, slot, 2 * hb : 2 * hb + 2, :],
                                            ident[:],
                                            is_transpose=True,
                                        )
                                    nc.scalar.activation(
                                        stm[:, hb, :], pt[:], Act.Copy
                                    )
                                if t == 0:
                                    pc = pcpool.tile([OC, CPIX], F32, name="pc")
                                for hb in range(2):
                                    for nb in range(CPIX // 512):
                                        nc.tensor.matmul(
                                            pc[:, 512 * nb : 512 * (nb + 1)],
                                            wpt[:, OC * t : OC * (t + 1)],
                                            stm[:, hb, 512 * nb : 512 * (nb + 1)],
                                            start=(t == 0 and hb == 0),
                                            stop=(t == KK - 1 and hb == 1),
                                        )
                    if "nope" in abl:
                        nc.gpsimd.dma_start(
                            out[0:64, CPIX * ch : CPIX * ch + 1024],
                            g[0:64, :, :, :].rearrange("p a b c -> p (a b c)")[
                                :, 0:1024
                            ],
                        )
                    else:
                        osb = opool.tile([OC, CPIX], F32, name="osb")
                        nc.scalar.activation(
                            osb[:], pc[:], Act.Identity, bias=bias[:, 0:1]
                        )
                        nc.sync.dma_start(
                            out[:, CPIX * ch : CPIX * (ch + 1)], osb[:]
                        )

            if loop_n > 0:
                with tc.For_i(0, loop_n, 1):
                    body()
            elif loop_n < 0:
                for _ in range(-loop_n):
                    body()
            else:
                body()
            if "gonly" in abl or "gonly2" in abl:
                dummy = cpool.tile([OC, 64], F32)
                nc.vector.memset(dummy[:], 1.0)
                nc.sync.dma_start(out[:, 0:64], dummy[:])

    nc.compile()
    return nc


def prep_core_inputs(x, offset, weight, bias, core):
    """Host-side shard/layout prep for one core. Pure layout, no math on
    tensor values (beyond the reference-mandated reshape semantics)."""
    s, half = core // 2, core % 2
    pad2 = os.environ.get("PAD", "2") == "2"
    xr = np.ascontiguousarray(x[s].transpose(1, 2, 0))          # [H, W, C]
    if pad2:
        # zero-framed image (+2 texels each side); pair-row (g0, j) holds
        # padded rows g0, g0+1 at padded col j.
        xf = np.zeros((H + 4, WP, C), np.float32)
        xf[2 : H + 2, 2 : W + 2] = xr
        rows = np.stack([xf[0:GP], xf[1 : GP + 1]], 1)   # [GP, 2, WP, C]
        xpair = np.concatenate(
            [rows.transpose(0, 2, 1, 3).reshape(GP * WP, 128),
             np.zeros((1, 128), np.float32)], 0)
    else:
        # interleaved row-pair NHWC: xpair[par*PIX + pr*128 + j] =
        #   [x[2pr+par, j, :], x[2pr+par+1, j, :]]
        xpad = np.concatenate([xr, np.zeros((1, W, C), np.float32)], 0)  # [129,W,C]
        rows = np.stack([xpad[0:128], xpad[1:129]], 1)           # [128, 2, W, C]
        # xpair row (g0, j) = [x[g0, j, :], x[g0+1, j, :]]
        xpair = np.concatenate(
            [rows.transpose(0, 2, 1, 3).reshape(2 * PIX, 128),
             np.zeros((1, 128), np.float32)], 0)

    # offsets for this half: [p=col, (slot, chan)] so the device load is
    # one contiguous row per partition
    off = np.ascontiguousarray(
        offset[s, :, 64 * half : 64 * half + HALF, :]
        .transpose(2, 1, 0)            # [W=128, HALF slots, 18]
        .reshape(128, SLOTS * 2 * KK)
    )
    # channel c=2t is x-offset, 2t+1 is y-offset (reference reshape
    # [kk,2,h,w]: x = off[:,:,0], y = off[:,:,1] -> channel t*2+0 / t*2+1)

    # tap-pair weight slabs [128, 5*64]: rows sub*64+c, cols pair*64+o
    wfull = weight.reshape(OC, C, KK)
    wpair = np.zeros((128, KK * OC), np.float32)
    for t in range(KK):
        wt = wfull[:, :, t].T          # [c, o]
        wpair[0:64, t * OC : (t + 1) * OC] = wt
        wpair[64:128, t * OC : (t + 1) * OC] = wt

    shift = 2.0 if pad2 else 0.0
    yy = np.broadcast_to(
        (np.arange(SLOTS, dtype=np.float32) + 64 * half + shift)[None, :],
        (128, SLOTS),
    ).copy()
    xxc = (np.arange(128, dtype=np.float32) + shift).reshape(128, 1).copy()
    return {
        "xpair": np.ascontiguousarray(xpair).astype(np.float16),
        "offs": off.astype(np.float32),
        "wpair": wpair.astype(np.float16),
        "yy": yy,
        "xx": xxc,
        "ident": np.eye(128, dtype=np.float16),
        "bias": bias.reshape(OC, 1).astype(np.float32),
    }


_CACHE = {}


def kernel(x, offset, weight, bias):
    x = np.asarray(x, np.float32)
    offset = np.asarray(offset, np.float32)
    weight = np.asarray(weight, np.float32)
    bias = np.asarray(bias, np.float32)
    if "nc" not in _CACHE:
        _CACHE["nc"] = build_program()
    nc = _CACHE["nc"]
    in_maps = [prep_core_inputs(x, offset, weight, bias, c) for c in range(8)]
    res = run_bass_kernel_spmd(nc, in_maps, core_ids=list(range(8)))
    outf = np.empty((B, OC, H, W), np.float32)
    for c in range(8):
        s, half = c // 2, c % 2
        outf[s, :, 64 * half : 64 * half + HALF, :] = res.results[c][
            "out"
        ].reshape(OC, HALF, W)
    return outf

